# revision 1
# baseline (speedup 1.0000x reference)
"""NRI-style GNN encoder (gnn_message_passing) on 8 Trainium2 NeuronCores.

Data-parallel over batch: core b computes batch element b end-to-end.

Math restructuring (exact, up to matmul dtype):
  - Edge-MLP first layers collapse: concat([r,s]) @ W = rec_gather(h @ Wa) +
    send_gather(h @ Wb).  Gathers are matmuls against the one-hot rel
    matrices (treated as data, not structure; one-hot is exact in fp16).
  - Activations are stored as z = elu(y) + 1; the "-1" folds into the
    consuming matmul's bias (b_eff = b - colsum(W)), precomputed on host.
  - elu(y) + 1 == min(exp(y), max(y + 1, 1))  (exact, incl. exp overflow).
  - Edge activations live transposed [feature(part), edge(free)] so chained
    layers need no transposes; only e1out needs both orientations for the
    aggregation matmul - done with fp16 PE transposes + one batched copy.
  - e1out (x_skip) is spilled to DRAM in fp16 and streamed back in pass 2.
  - Edge passes are software-pipelined at emission: stage B (2nd layer) lags
    one macro behind stage A, aggregation lags two - so no engine queue ever
    head-of-line blocks on a cross-engine chain.
"""

import os
import sys

for _p in ("/opt/trn_rl_repo",):
    if _p not in sys.path:
        sys.path.insert(0, _p)

import numpy as np
import ml_dtypes

import concourse.bass as bass
import concourse.tile as tile
from concourse import bacc, mybir
from concourse.bass_utils import run_bass_kernel_spmd

DT = mybir.dt
AF = mybir.ActivationFunctionType
ALU = mybir.AluOpType

B, N, T, D, H, NE = 8, 128, 49, 4, 256, 2
E = N * (N - 1)          # 16256
F = T * D                # 196
MACRO = 512              # edges per macro-tile
MM = 512                 # matmul moving chunk (psum-bank limit for fp32 out)


# packed-constant column layouts: name -> (col_offset, width)
def _mk_layout(entries):
    out, c = {}, 0
    for name, w in entries:
        out[name] = (c, w)
        c += w
    return out, c

PK32, C32 = _mk_layout([
    ("ey32", 128), ("wn1a", 256), ("wn1b", 256),
    ("wn1l2", 512), ("a1s", 512), ("b1s", 512),
    ("wn2l1", 512), ("wn2l2", 512), ("a2s", 512), ("b2s", 512),
    ("bpk", 24), ("nbs", 1024), ("bos", 16), ("ows", 4),
    ("ones1", 128), ("be1r", 256), ("be3r", 256),
])
PK16, C16 = _mk_layout([
    ("ey16", 128), ("we1l2", 512), ("c2s", 512), ("we2l2", 512),
    ("ones16", 512), ("be2r", 256), ("be4r", 256),
])

_PROG_CACHE = {}
LAST_EXEC_NS = None


def _build_program():
    nc = bacc.Bacc(
        "TRN2",
        target_bir_lowering=False,
        debug=False,
        enable_asserts=True,
        num_devices=8,
    )

    f32, f16, f32r = DT.float32, DT.float16, DT.float32r

    def din(name, shape, dt=f32):
        return nc.dram_tensor(name, list(shape), dt, kind="ExternalInput").ap()

    # ---- DRAM I/O ----
    x_in = din("x_nm", [N, F])                     # per-core batch slice
    recT = din("recT", [N, E], f16)                # rec_rel.T (one-hot, exact)
    sendT = din("sendT", [N, E], f16)              # send_rel.T
    recN = din("recN", [E, N], f16)                # rec_rel (native)

    # all constants packed into two blobs (one DMA each); column layout
    # must match _prep_inputs
    pk32 = din("pk32", [128, C32], f32)
    pk16 = din("pk16", [128, C16], f16)

    out_d = nc.dram_tensor("out", [E, NE], f32, kind="ExternalOutput").ap()

    offs = list(range(0, E, MACRO))

    def sq(w):  # view [256, x] dram as [128, 2, x] (partition-major halves)
        return w.rearrange("(h p) o -> p h o", p=128)

    with tile.TileContext(nc) as tc:
        with (
            tc.tile_pool(name="const", bufs=1) as cpool,
            tc.tile_pool(name="rel", bufs=1) as relpool,
            tc.tile_pool(name="work", bufs=6) as wk,
            tc.tile_pool(name="zebuf", bufs=4) as zb,
            tc.tile_pool(name="dspill", bufs=1, space="DRAM") as dsp,
            tc.tile_pool(name="pre_ps", bufs=2, space="PSUM") as pre_ps,
            tc.tile_pool(name="l2_ps", bufs=2, space="PSUM") as l2_ps,
        ):
            # ---------- load constants ----------
            def ctile(ap_dram, shape, dt=f32, name="c"):
                t = cpool.tile(shape, dt, name=name)
                nc.sync.dma_start(t[:], ap_dram)
                return t

            x_sb = ctile(x_in, [N, F], name="x_sb")
            p32 = ctile(pk32, [128, C32], f32, name="p32")
            p16 = ctile(pk16, [128, C16], f16, name="p16")

            def c32(name, hview=False, f32v=False):
                c0, w = PK32[name]
                ap = p32[:, c0:c0 + w]
                if hview:
                    ap = ap.rearrange("p (h o) -> p h o", h=2)
                return ap

            def c16(name, hview=False):
                c0, w = PK16[name]
                ap = p16[:, c0:c0 + w]
                if hview:
                    ap = ap.rearrange("p (h o) -> p h o", h=2)
                return ap

            ey32 = c32("ey32", f32v=True)
            ey16 = c16("ey16")
            wn1a = c32("wn1a")
            wn1b = c32("wn1b")[0:68, :]
            wn1l2 = c32("wn1l2", hview=True)
            a1s = c32("a1s", hview=True)
            b1s = c32("b1s", hview=True)
            we1l2 = c16("we1l2", hview=True)
            wn2l1 = c32("wn2l1", hview=True)
            wn2l2 = c32("wn2l2", hview=True)
            a2s = c32("a2s", hview=True)
            b2s = c32("b2s", hview=True)
            c2s = c16("c2s", hview=True)
            we2l2 = c16("we2l2", hview=True)
            ows = c32("ows", hview=True)
            bpk = c32("bpk", f32v=True)
            nbs = c32("nbs", f32v=True).rearrange("p (h o) -> p h o", h=4)
            bos = c32("bos", f32v=True)
            ones1 = c32("ones1")[0:1, :]
            ones16 = c16("ones16")[0:1, :]
            be2r = c16("be2r")[0:1, :]
            be4r = c16("be4r")[0:1, :]
            be1r = c32("be1r")[0:1, :]
            be3r = c32("be3r")[0:1, :]

            # rel matrices resident in SBUF (fp16: 32.5KB/part each),
            # loaded in per-macro slices to overlap with compute
            recT_sb = relpool.tile([128, E], f16, name="recT_sb")
            sendT_sb = relpool.tile([128, E], f16, name="sendT_sb")
            bounds = [0, 1024, 2048, 4064, 8128, 12192, E]
            for c0, c1 in zip(bounds[:-1], bounds[1:]):
                nc.sync.dma_start(recT_sb[:, c0:c1], recT[:, c0:c1])
                nc.sync.dma_start(sendT_sb[:, c0:c1], sendT[:, c0:c1])

            # e1out spill (fp16) in DRAM
            ze1f = dsp.tile([128, 2, E], f16, name="ze1f")

            def bcol(c):
                return bpk[:, c:c + 1]

            # ---------- helpers ----------
            # bias_pk columns: per stage s in 0..3: [6s + fh]: b
            #                                      [6s + 2 + fh]: b + 1
            #                                      [6s + 4 + fh]: -b  (D2 relu)
            def elu_T(ps_ap, stage, fh, out_ap, L, form, hi=False):
                """Transposed-layout ELU(+1).

                form "D1": t=Exp(y+b) [ACT]; r=max(y+b+1,1) [DVE TS];
                           out=min(t,r) [DVE TT]
                form "D2": t=Exp(y+b) [ACT]; r0=Relu(y+b) [ACT];
                           out=(r0+1) min t [DVE STT]
                hi=True keeps t/r in fp32 (for the last stage, whose rounding
                dominates the output error).
                """
                if hi:
                    t = wk.tile([128, MACRO], f32, name="t_exp32", tag="t_exp32",
                                bufs=3)
                    r = wk.tile([128, MACRO], f32, name="r_max32", tag="r_max32",
                                bufs=3)
                    nc.scalar.activation(t[:, :L], ps_ap, AF.Exp,
                                         bias=bcol(6 * stage + fh))
                    if form == "D2":
                        nc.scalar.activation(r[:, :L], ps_ap, AF.Relu,
                                             bias=bcol(6 * stage + fh))
                        nc.vector.scalar_tensor_tensor(out_ap, r[:, :L], 1.0,
                                                       t[:, :L], ALU.add,
                                                       ALU.min)
                    else:
                        nc.vector.tensor_scalar(r[:, :L], ps_ap,
                                                bcol(6 * stage + 2 + fh), 1.0,
                                                ALU.add, ALU.max)
                        nc.vector.tensor_tensor(out_ap, t[:, :L], r[:, :L],
                                                ALU.min)
                    return
                t = wk.tile([128, MACRO], f16, name="t_exp", tag="t_exp",
                            bufs=4)
                nc.scalar.activation(t[:, :L], ps_ap, AF.Exp,
                                     bias=bcol(6 * stage + fh))
                if form == "D1":
                    r = wk.tile([128, MACRO], f16, name="r_max", tag="r_max",
                                bufs=4)
                    nc.vector.tensor_scalar(r[:, :L], ps_ap,
                                            bcol(6 * stage + 2 + fh), 1.0,
                                            ALU.add, ALU.max)
                    nc.vector.tensor_tensor(out_ap, t[:, :L], r[:, :L], ALU.min)
                else:
                    r = wk.tile([128, MACRO], f16, name="r_max", tag="r_max",
                                bufs=4)
                    nc.scalar.activation(r[:, :L], ps_ap, AF.Relu,
                                         bias=bcol(6 * stage + fh))
                    nc.vector.scalar_tensor_tensor(out_ap, r[:, :L], 1.0,
                                                   t[:, :L], ALU.add, ALU.min)

            def elu_N(y_sb, out_name):
                """Node-layout ELU(+1) on [128, 256] sbuf (bias already added)."""
                t = wk.tile([128, 256], f32, name="t_n", tag="t_n")
                nc.scalar.activation(t[:], y_sb, AF.Exp)
                r = wk.tile([128, 256], f32, name="r_n", tag="r_n")
                nc.vector.tensor_scalar(r[:], y_sb, 1.0, 1.0, ALU.add, ALU.max)
                z = cpool.tile([128, 256], f32, name=out_name)
                nc.vector.tensor_tensor(z[:], t[:], r[:], ALU.min)
                return z

            def tpose_nf(src_sb, out_name):
                """[128n, 256f] sbuf -> [128f-local, 2(fh), 128n] sbuf."""
                ps = l2_ps.tile([128, MACRO], f32, name="ps_tp", tag="l2")
                for fh in range(2):
                    nc.tensor.transpose(ps[:, fh * 128:(fh + 1) * 128],
                                        src_sb[:, fh * 128:(fh + 1) * 128],
                                        ey32)
                t = cpool.tile([128, 2, 128], f32, name=out_name)
                nc.vector.tensor_copy(t[:].rearrange("p a b -> p (a b)"),
                                      ps[:, :256])
                return t

            def node_mm(lhsT_tile, rhs_tile, nh=2, brow=None, rows=()):
                """sum_fh lhsT[:, fh].T @ rhs[:, fh] (+ K=1 row mms) -> psum."""
                if brow is not None:
                    rows = ((ones1, brow),) + tuple(rows)
                ps = pre_ps.tile([128, MACRO], f32, name="ps_n", tag="pre")
                for fh in range(nh):
                    nc.tensor.matmul(ps[:, :256], lhsT_tile[:, fh],
                                     rhs_tile[:, fh],
                                     start=(fh == 0),
                                     stop=(fh == nh - 1 and not rows))
                for i, (lr, rr) in enumerate(rows):
                    nc.tensor.matmul(ps[:, :256], lr, rr,
                                     start=False, stop=(i == len(rows) - 1))
                return ps

            def add_bias_sbuf(ps, btile, name):
                y = wk.tile([128, 256], f32, name=name, tag="y_n")
                nc.vector.tensor_tensor(y[:], ps[:, :256], btile, ALU.add)
                return y

            def copy16(ps, name):
                u = cpool.tile([128, 256], f16, name=name)
                nc.scalar.copy(u[:], ps[:, :256])
                return u

            def elu_nb(ps_ap, out_ap, FD, form, hi=False):
                """ELU(+1) with bias already in psum (immediate scalars)."""
                if hi:
                    t = wk.tile([128, 2 * MACRO], f32, name="t_exp32",
                                tag="t_exp32", bufs=2)
                    r = wk.tile([128, 2 * MACRO], f32, name="r_max32",
                                tag="r_max32", bufs=2)
                else:
                    t = wk.tile([128, 2 * MACRO], f16, name="t_exp",
                                tag="t_exp", bufs=4)
                    r = wk.tile([128, 2 * MACRO], f16, name="r_max",
                                tag="r_max", bufs=4)
                nc.scalar.activation(t[:, :FD], ps_ap, AF.Exp)
                if form == "D1":
                    nc.vector.tensor_scalar(r[:, :FD], ps_ap, 1.0, 1.0,
                                            ALU.add, ALU.max)
                    nc.vector.tensor_tensor(out_ap, t[:, :FD], r[:, :FD],
                                            ALU.min)
                else:
                    nc.scalar.activation(r[:, :FD], ps_ap, AF.Relu)
                    nc.vector.scalar_tensor_tensor(out_ap, r[:, :FD], 1.0,
                                                   t[:, :FD], ALU.add, ALU.min)

            # ---------- node stage 1 ----------
            ps_x = l2_ps.tile([128, MACRO], f32, name="ps_x", tag="l2")
            nc.tensor.transpose(ps_x[:, 0:128], x_sb[:, 0:128], ey32)
            nc.tensor.transpose(ps_x[0:68, 128:256], x_sb[:, 128:196], ey32)
            xt0 = cpool.tile([128, 128], f32, name="xt0")
            nc.vector.tensor_copy(xt0[:], ps_x[:, 0:128])
            xt1 = cpool.tile([68, 128], f32, name="xt1")
            nc.vector.tensor_copy(xt1[:], ps_x[0:68, 128:256])

            ps1 = pre_ps.tile([128, MACRO], f32, name="ps1", tag="pre")
            nc.tensor.matmul(ps1[:, :256], xt0[:], wn1a[:],
                             start=True, stop=False)
            nc.tensor.matmul(ps1[:, :256], xt1[:], wn1b[:],
                             start=False, stop=True)
            y1 = add_bias_sbuf(ps1, nbs[:, 0, :], "y1")
            zh1a = elu_N(y1[:], "zh1a")
            zh1aT = tpose_nf(zh1a, "zh1aT")

            ps2 = node_mm(zh1aT, wn1l2)
            y2 = add_bias_sbuf(ps2, nbs[:, 1, :], "y2")
            zh1 = elu_N(y2[:], "zh1")
            zh1T = tpose_nf(zh1, "zh1T")

            u1 = copy16(node_mm(zh1T, a1s, brow=be1r), "u1")
            v1 = copy16(node_mm(zh1T, b1s), "v1")

            # ---------- pass 1 over edges (software-pipelined) ----------
            def p1_stageA(off, L, mi):
                """e1pre gather matmuls + fused 2-half ELU -> ze1a (fp16)."""
                ze1a = zb.tile([128, 2, MACRO], f16, name="ze1a", tag="ze1a")
                ps = pre_ps.tile([128, 2, MACRO], f32, name="ps_p1", tag="pre")
                for fh in range(2):
                    nc.tensor.matmul(
                        ps[:, fh, :L], u1[:, fh * 128:(fh + 1) * 128],
                        recT_sb[:, off:off + L], start=True, stop=False)
                    nc.tensor.matmul(
                        ps[:, fh, :L], v1[:, fh * 128:(fh + 1) * 128],
                        sendT_sb[:, off:off + L], start=False, stop=True)
                if L == MACRO:
                    elu_nb(ps[:].rearrange("p a b -> p (a b)"),
                           ze1a[:].rearrange("p a b -> p (a b)"), 2 * L,
                           "D1" if mi % 4 == 0 else "D2")
                else:
                    for fh in range(2):
                        elu_nb(ps[:, fh, :L], ze1a[:, fh, :L], L, "D1")
                return ze1a

            def p1_stageB(off, L, ze1a, mi):
                """e1l2 matmuls + ELU -> ze1 (fp16); spill; DMA-transpose."""
                ze1 = zb.tile([128, 2, MACRO], f16, name="ze1", tag="ze1")
                for oh in range(2):
                    ps = l2_ps.tile([128, MACRO], f32, name="ps_l1", tag="l2")
                    for fh in range(2):
                        nc.tensor.matmul(
                            ps[:, :L],
                            we1l2[:, fh, oh * 128:(oh + 1) * 128],
                            ze1a[:, fh, :L],
                            start=(fh == 0), stop=(fh == 1))
                    elu_T(ps[:, :L], 1, oh, ze1[:, oh, :L], L,
                          "D2" if (oh == 0 and mi % 2 == 0) else "D1")
                nc.sync.dma_start(ze1f[:, :, off:off + L], ze1[:, :, :L])
                nsub = L // 128
                tp = tp_ps.tile([128, 4, 256], f16, name="tp", tag="tp")
                for j in range(nsub):
                    for fh in range(2):
                        nc.tensor.transpose(
                            tp[:, j, fh * 128:(fh + 1) * 128],
                            ze1[:, fh, j * 128:(j + 1) * 128],
                            ey16)
                zunt = wk.tile([128, 4, 256], f16, name="zunt", tag="zunt",
                               bufs=4)
                nc.vector.tensor_copy(
                    zunt[:, :nsub, :].rearrange("p a b -> p (a b)"),
                    tp[:, :nsub, :].rearrange("p a b -> p (a b)"))
                return zunt

            def p1_stageC(aggp, rN, zunt, nsub, sub_base, n_sub_total):
                for j in range(nsub):
                    si = sub_base + j
                    for fh in range(2):
                        nc.tensor.matmul(
                            aggp[:, fh, :],
                            zunt[:, j, fh * 128:(fh + 1) * 128],
                            rN[:, j, :],
                            start=(si == 0 and fh == 0),
                            stop=(si == n_sub_total - 1 and fh == 1),
                            skip_group_check=True)

            with (
                tc.tile_pool(name="agg_ps", bufs=1, space="PSUM") as agg_ps,
                tc.tile_pool(name="tp_ps", bufs=1, space="PSUM") as tp_ps,
            ):
                aggp = agg_ps.tile([128, 2, 128], f32, name="aggp")
                n_sub_total = E // 128  # 127

                recs = []
                for off in offs:
                    L = min(MACRO, E - off)
                    rN = wk.tile([128, 4, 128], f16, name="rN",
                                 tag="rN", bufs=4)
                    nc.sync.dma_start(
                        rN[:, :L // 128, :],
                        recN[off:off + L, :].rearrange("(j p) n -> p j n",
                                                       p=128))
                    ze1a = p1_stageA(off, L, off // MACRO)
                    recs.append(dict(off=off, L=L, rN=rN, ze1a=ze1a,
                                     zunts=None, sub=off // 128))
                    i = len(recs) - 1
                    if i >= 1:
                        r = recs[i - 1]
                        r["zunts"] = p1_stageB(r["off"], r["L"], r["ze1a"],
                                               i - 1)
                    if i >= 2:
                        r = recs[i - 2]
                        p1_stageC(aggp, r["rN"], r["zunts"], r["L"] // 128,
                                  r["sub"], n_sub_total)
                r = recs[-1]
                r["zunts"] = p1_stageB(r["off"], r["L"], r["ze1a"], len(recs) - 1)
                for r in recs[-2:]:
                    p1_stageC(aggp, r["rN"], r["zunts"], r["L"] // 128,
                              r["sub"], n_sub_total)

                # ---------- node stage 2 ----------
                aggT = cpool.tile([128, 2, 128], f32, name="aggT")
                nc.scalar.copy(aggT[:].rearrange("p a b -> p (a b)"),
                               aggp[:].rearrange("p a b -> p (a b)"))

            ps3 = node_mm(aggT, wn2l1)
            y3 = add_bias_sbuf(ps3, nbs[:, 2, :], "y3")
            zh2a = elu_N(y3[:], "zh2a")
            zh2aT = tpose_nf(zh2a, "zh2aT")

            ps4 = node_mm(zh2aT, wn2l2)
            y4 = add_bias_sbuf(ps4, nbs[:, 3, :], "y4")
            zh2 = elu_N(y4[:], "zh2")
            zh2T = tpose_nf(zh2, "zh2T")

            u2 = copy16(node_mm(zh2T, a2s, brow=be3r), "u2")
            v2 = copy16(node_mm(zh2T, b2s), "v2")

            # ---------- pass 2 over edges (software-pipelined) ----------
            def p2_stageA(off, L, zskip, mi):
                ze2a = zb.tile([128, 2, MACRO], f16, name="ze2a", tag="ze2a")
                ps = pre_ps.tile([128, 2, MACRO], f32, name="ps_p2", tag="pre")
                for fh in range(2):
                    # skip-term first: depends only on the pass-1 spill, so
                    # the PE can run it while node stage 2 is still serial
                    for hh in range(2):
                        nc.tensor.matmul(
                            ps[:, fh, :L],
                            c2s[:, hh, fh * 128:(fh + 1) * 128],
                            zskip[:, hh, :L],
                            start=(hh == 0), stop=False)
                    nc.tensor.matmul(
                        ps[:, fh, :L], u2[:, fh * 128:(fh + 1) * 128],
                        recT_sb[:, off:off + L], start=False, stop=False)
                    nc.tensor.matmul(
                        ps[:, fh, :L], v2[:, fh * 128:(fh + 1) * 128],
                        sendT_sb[:, off:off + L], start=False, stop=True)
                if L == MACRO:
                    elu_nb(ps[:].rearrange("p a b -> p (a b)"),
                           ze2a[:].rearrange("p a b -> p (a b)"), 2 * L,
                           "D1" if mi % 4 == 0 else "D2")
                else:
                    for fh in range(2):
                        elu_nb(ps[:, fh, :L], ze2a[:, fh, :L], L, "D1")
                return ze2a

            def p2_stageB(off, L, ze2a):
                ze2t = zb.tile([128, 2, MACRO], f32, name="ze2t", tag="ze2t",
                               bufs=3)
                for oh in range(2):
                    ps = l2_ps.tile([128, MACRO], f32, name="ps_l2", tag="l2")
                    for fh in range(2):
                        nc.tensor.matmul(
                            ps[:, :L],
                            we2l2[:, fh, oh * 128:(oh + 1) * 128],
                            ze2a[:, fh, :L],
                            start=(fh == 0), stop=(fh == 1))
                    elu_T(ps[:, :L], 3, oh, ze2t[:, oh, :L], L,
                          "D2" if (oh == 0 and off // MACRO % 2 == 0) else "D1",
                          hi=True)
                nsub = L // 128
                op = out_ps.tile([128, 16], f32, name="op", tag="op")
                for j in range(nsub):
                    for hh in range(2):
                        nc.tensor.matmul(
                            op[:, 2 * j:2 * j + 2],
                            ze2t[:, hh, j * 128:(j + 1) * 128],
                            ows[:, hh, :],
                            start=(hh == 0), stop=(hh == 1))
                osb = wk.tile([128, 16], f32, name="osb", tag="osb")
                nc.vector.tensor_tensor(osb[:, :2 * nsub], op[:, :2 * nsub],
                                        bos[:, :2 * nsub], ALU.add)
                nc.sync.dma_start(
                    out_d[off:off + L, :].rearrange("(j p) c -> p j c", p=128),
                    osb[:, :2 * nsub].rearrange("p (j c) -> p j c", c=NE))

            with tc.tile_pool(name="out_ps", bufs=2, space="PSUM") as out_ps:
                recs2 = []
                for off in offs:
                    L = min(MACRO, E - off)
                    zskip = wk.tile([128, 2, MACRO], f16, name="zskip",
                                    tag="zskip", bufs=2)
                    nc.sync.dma_start(zskip[:, :, :L], ze1f[:, :, off:off + L])
                    ze2a = p2_stageA(off, L, zskip, off // MACRO)
                    recs2.append(dict(off=off, L=L, ze2a=ze2a))
                    if len(recs2) >= 2:
                        r = recs2[-2]
                        p2_stageB(r["off"], r["L"], r["ze2a"])
                r = recs2[-1]
                p2_stageB(r["off"], r["L"], r["ze2a"])

    nc.compile()
    return nc


def _prep_inputs(inputs):
    """Host-side constant preprocessing -> shared in_map (all cores)."""
    f = lambda a: np.ascontiguousarray(np.asarray(a, dtype=np.float32))
    rec, send = f(inputs["rec_rel"]), f(inputs["send_rel"])
    cs = lambda w: w.sum(axis=0)

    n1w1, n1b1 = f(inputs["n1w1"]), f(inputs["n1b1"])
    n1w2, n1b2 = f(inputs["n1w2"]), f(inputs["n1b2"])
    e1w1, e1b1 = f(inputs["e1w1"]), f(inputs["e1b1"])
    e1w2, e1b2 = f(inputs["e1w2"]), f(inputs["e1b2"])
    n2w1, n2b1 = f(inputs["n2w1"]), f(inputs["n2b1"])
    n2w2, n2b2 = f(inputs["n2w2"]), f(inputs["n2b2"])
    e2w1, e2b1 = f(inputs["e2w1"]), f(inputs["e2b1"])
    e2w2, e2b2 = f(inputs["e2w2"]), f(inputs["e2b2"])
    ow, ob = f(inputs["ow"]), f(inputs["ob"])

    A1, B1 = e1w1[:256], e1w1[256:]
    A2, B2, C2 = e2w1[:256], e2w1[256:512], e2w1[512:]

    e1w2_h = e1w2.astype(np.float16)
    C2_h = C2.astype(np.float16)
    e2w2_h = e2w2.astype(np.float16)

    be1 = e1b1 - cs(A1) - cs(B1)
    be2 = e1b2 - cs(e1w2_h.astype(np.float32))
    be3 = e2b1 - cs(A2) - cs(B2) - cs(C2_h.astype(np.float32))
    be4 = e2b2 - cs(e2w2_h.astype(np.float32))
    ob_adj = ob - cs(ow)

    bias_pk = np.zeros((128, 24), np.float32)
    for i, v in enumerate((be1, be2, be3, be4)):
        vv = v.reshape(2, 128)
        for fh in range(2):
            bias_pk[:, 6 * i + fh] = vv[fh]
            bias_pk[:, 6 * i + 2 + fh] = vv[fh] + 1.0
            bias_pk[:, 6 * i + 4 + fh] = -vv[fh]

    indeg = rec.sum(axis=0)  # [N]
    nbias = np.zeros((128, 4, 256), np.float32)
    nbias[:, 0, :] = n1b1[None, :]
    nbias[:, 1, :] = (n1b2 - cs(n1w2))[None, :]
    nbias[:, 2, :] = n2b1[None, :] - indeg[:, None] * cs(n2w1)[None, :]
    nbias[:, 3, :] = (n2b2 - cs(n2w2))[None, :]

    bout = np.tile(ob_adj[None, :], (128, 8)).astype(np.float32)

    def sqh(w):  # [256, x] -> [128, 2*x] partition-major halves
        return np.ascontiguousarray(
            w.reshape(2, 128, -1).transpose(1, 0, 2).reshape(128, -1))

    pk32 = np.zeros((128, C32), np.float32)
    def put32(name, arr):
        c0, w = PK32[name]
        pk32[:arr.shape[0], c0:c0 + w] = arr
    put32("ey32", np.eye(128, dtype=np.float32))
    put32("wn1a", n1w1[:128])
    put32("wn1b", n1w1[128:])
    put32("wn1l2", sqh(n1w2))
    put32("a1s", sqh(A1)); put32("b1s", sqh(B1))
    put32("wn2l1", sqh(n2w1)); put32("wn2l2", sqh(n2w2))
    put32("a2s", sqh(A2)); put32("b2s", sqh(B2))
    put32("bpk", bias_pk)
    put32("nbs", nbias.reshape(128, -1))
    put32("bos", bout)
    put32("ows", sqh(ow))
    c0, w = PK32["ones1"]; pk32[0, c0:c0 + w] = 1.0
    c0, w = PK32["be1r"]; pk32[0, c0:c0 + w] = be1
    c0, w = PK32["be3r"]; pk32[0, c0:c0 + w] = be3

    pk16 = np.zeros((128, C16), np.float16)
    def put16(name, arr):
        c0, w = PK16[name]
        pk16[:arr.shape[0], c0:c0 + w] = arr
    put16("ey16", np.eye(128, dtype=np.float16))
    put16("we1l2", sqh(e1w2_h.astype(np.float32)).astype(np.float16))
    put16("c2s", sqh(C2_h.astype(np.float32)).astype(np.float16))
    put16("we2l2", sqh(e2w2_h.astype(np.float32)).astype(np.float16))
    c0, w = PK16["ones16"]; pk16[0, c0:c0 + w] = 1.0
    c0, w = PK16["be2r"]; pk16[0, c0:c0 + w] = be2.astype(np.float16)
    c0, w = PK16["be4r"]; pk16[0, c0:c0 + w] = be4.astype(np.float16)

    shared = dict(
        recT=np.ascontiguousarray(rec.T.astype(np.float16)),
        sendT=np.ascontiguousarray(send.T.astype(np.float16)),
        recN=np.ascontiguousarray(rec.astype(np.float16)),
        pk32=pk32, pk16=pk16,
    )
    return shared


def kernel(**inputs):
    global LAST_EXEC_NS
    if "prog" not in _PROG_CACHE:
        _PROG_CACHE["prog"] = _build_program()
    nc = _PROG_CACHE["prog"]

    shared = _prep_inputs(inputs)
    x = np.asarray(inputs["x"], dtype=np.float32)
    in_maps = []
    for b in range(B):
        m = dict(shared)
        m["x_nm"] = np.ascontiguousarray(x[b].reshape(N, F))
        in_maps.append(m)

    trace = os.environ.get("KERNEL_TRACE", "0") == "1"
    try:
        res = run_bass_kernel_spmd(nc, in_maps, core_ids=list(range(8)),
                                   trace=trace)
    except ModuleNotFoundError:
        # NTFF profiling hook unavailable in this environment
        res = run_bass_kernel_spmd(nc, in_maps, core_ids=list(range(8)),
                                   trace=False)
    if trace and res.exec_time_ns is not None:
        LAST_EXEC_NS = res.exec_time_ns
        print(f"HW exec time: {res.exec_time_ns} ns "
              f"(mean {res.mean_exec_time_ns} ns, "
              f"slowest core {res.max_exec_time_core_id})")

    out = np.stack([res.results[b]["out"] for b in range(B)], axis=0)
    return out.astype(np.float32)

